# revision 1
# baseline (speedup 1.0000x reference)
"""DeformConv2d Trainium2 kernel: batch-parallel over 8 NeuronCores."""
import sys
sys.path.insert(0, '/opt/trn_rl_repo')
import numpy as np

import concourse.bass as bass
from concourse import bacc, mybir
from concourse.alu_op_type import AluOpType
from concourse.tile import TileContext
from concourse.bass_utils import run_bass_kernel_spmd

F16 = mybir.dt.float16
F32 = mybir.dt.float32
I16 = mybir.dt.int16

H = W = 56
C = 64
OC = 64
KK = 9            # taps
P = H * W         # 3136
PPAD = 3200       # padded positions (quarters 896*3+512)
HALF = 1664  # unused
NHALF = 2
CHUNKS = [(0, 512), (512, 512), (1024, 512), (1536, 128)]  # (start, len) within a half
GW = 58           # haloed grid width  (y,x in [-1,56])
TVIEW = 3392      # overlapped-row view rows
TROWS = 3456      # physical T2 rows
NCOL = PPAD // 16  # 208 idx columns per tap
NCOL_REAL = P // 16  # 196

# ---------------- host prep ----------------

def build_t2(x_img: np.ndarray) -> np.ndarray:
    """x_img [C,H,W] f32 -> T2 [TROWS,128] f16 pair table with zero halo."""
    xh = x_img.astype(np.float16)
    t2 = np.zeros((TROWS, 128), dtype=np.float16)
    grid = t2[:GW * GW].reshape(GW, GW, 128)
    # cell g=(gy,gx) covers (y,x)=(gy-1,gx-1); top half holds value at (y,x)
    grid[1:57, 1:57, 0:C] = xh.transpose(1, 2, 0)
    # bottom half holds value at (y+1, x): row y+1 = gy, valid gy<=56 -> y+1 in [0,55] means gy in [0,55]
    grid[0:56, 1:57, C:2 * C] = xh.transpose(1, 2, 0)
    return t2


def _grids_f32():
    ky, kx = np.meshgrid(np.arange(3), np.arange(3), indexing='ij')
    ky = ky.reshape(KK)
    kx = kx.reshape(KK)
    oy, ox = np.meshgrid(np.arange(H), np.arange(W), indexing='ij')
    gy = (oy[None] - 1 + ky[:, None, None] + 8).astype(np.float32)  # [K,H,W]
    gx = (ox[None] - 1 + kx[:, None, None] + 8).astype(np.float32)
    return gy.reshape(KK, P), gx.reshape(KK, P)


def nat_layout(planes_y, planes_x):
    out = np.full((128, 448), 8.0, dtype=np.float32)
    out[0:63] = planes_y.reshape(KK * 7, 448)
    out[64:127] = planes_x.reshape(KK * 7, 448)
    return out


def wrap_layout(planes_y, planes_x):
    def wrap1(pl):  # [P] -> [16, 196]
        return pl.reshape(NCOL_REAL, 16).T.copy()
    A = np.empty((128, 392), dtype=np.float32)
    B = np.empty((16, 392), dtype=np.float32)
    for k in range(8):
        A[16 * k:16 * k + 16, 0:196] = wrap1(planes_y[k])
        A[16 * k:16 * k + 16, 196:392] = wrap1(planes_x[k])
    B[:, 0:196] = wrap1(planes_y[8])
    B[:, 196:392] = wrap1(planes_x[8])
    return A, B


def _natmask():
    m = np.ones((128, 448), dtype=np.float32)
    m[63] = 0.0
    m[127] = 0.0
    return m


def host_inputs(x_img, off_img, weight):
    gy, gx = _grids_f32()
    offp = off_img.reshape(KK, 2, P)
    wt = weight.reshape(OC, C, KK).transpose(1, 2, 0)  # [C, K, OC]
    wlhs = np.concatenate([wt, wt], axis=0).astype(np.float16)  # [128, K, OC]
    rlhs = np.zeros((2, 128), dtype=np.float16)
    rlhs[0, 0:64] = 1.0
    rlhs[1, 64:128] = 1.0
    ins = {
        "t2": build_t2(x_img),
        "off_nat": nat_layout(offp[:, 0], offp[:, 1]) * _natmask(),
        "grid_nat": nat_layout(gy, gx),
        "wlhs": np.ascontiguousarray(wlhs),
        "rlhs": rlhs,
    }
    wa, wb = wrap_layout(offp[:, 0], offp[:, 1])
    ga, gb = wrap_layout(gy, gx)
    ins["off_wa"], ins["off_wb"] = wa, wb
    ins["grid_wa"], ins["grid_wb"] = ga, gb
    return ins


# ---------------- device kernel ----------------

def gen_kernel(n_cores=8, loop_n=None):
    nc = bacc.Bacc("TRN2", target_bir_lowering=False, debug=False, num_devices=n_cores)

    t2 = nc.dram_tensor("t2", [TROWS, 128], F16, kind="ExternalInput")
    off_nat = nc.dram_tensor("off_nat", [128, 448], F32, kind="ExternalInput")
    grid_nat = nc.dram_tensor("grid_nat", [128, 448], F32, kind="ExternalInput")
    off_wa = nc.dram_tensor("off_wa", [128, 392], F32, kind="ExternalInput")
    off_wb = nc.dram_tensor("off_wb", [16, 392], F32, kind="ExternalInput")
    grid_wa = nc.dram_tensor("grid_wa", [128, 392], F32, kind="ExternalInput")
    grid_wb = nc.dram_tensor("grid_wb", [16, 392], F32, kind="ExternalInput")
    wlhs = nc.dram_tensor("wlhs", [128, KK, OC], F16, kind="ExternalInput")
    rlhs = nc.dram_tensor("rlhs", [2, 128], F16, kind="ExternalInput")
    out = nc.dram_tensor("out", [OC, P], F32, kind="ExternalOutput")

    with TileContext(nc) as tc:
        with tc.tile_pool(name="const", bufs=1) as const, \
             tc.tile_pool(name="pipe", bufs=1) as pipe, \
             tc.tile_pool(name="gpool", bufs=12) as gpool, \
             tc.tile_pool(name="upool", bufs=6) as upool, \
             tc.tile_pool(name="wtpool", bufs=6) as wtpool, \
             tc.tile_pool(name="opool", bufs=2) as opool, \
             tc.tile_pool(name="psw", bufs=2, space="PSUM") as psw, \
             tc.tile_pool(name="pso", bufs=1, space="PSUM") as pso:

            def ctile(shape, dt, tag):
                return const.tile(shape, dt, tag=tag, name=tag)

            # ---- load constants / inputs ----
            wlhs_sb = ctile([128, KK, OC], F16, "wlhs_sb")
            nc.sync.dma_start(out=wlhs_sb[:], in_=wlhs.ap())
            rlhs_sb = ctile([2, 128], F16, "rlhs_sb")
            nc.sync.dma_start(out=rlhs_sb[:], in_=rlhs.ap())

            onat = ctile([128, 448], F32, "onat")
            nc.sync.dma_start(out=onat[:], in_=off_nat.ap())
            gnat = ctile([128, 448], F32, "gnat")
            nc.sync.dma_start(out=gnat[:], in_=grid_nat.ap())
            owa = ctile([128, 392], F32, "owa")
            nc.sync.dma_start(out=owa[:], in_=off_wa.ap())
            gwa = ctile([128, 392], F32, "gwa")
            nc.sync.dma_start(out=gwa[:], in_=grid_wa.ap())
            owb = ctile([16, 392], F32, "owb")
            nc.sync.dma_start(out=owb[:], in_=off_wb.ap())
            gwb = ctile([16, 392], F32, "gwb")
            nc.sync.dma_start(out=gwb[:], in_=grid_wb.ap())

            loop_ctx = tc.For_i(0, loop_n, 1) if loop_n else None
            import contextlib
            with (loop_ctx if loop_ctx is not None else contextlib.nullcontext()):
                def pt(tag, shape=(128, 448), dt=F32):
                    return pipe.tile(list(shape), dt, tag=tag, name=tag)

                # ---- wrapped-layout index pipeline ----
                def idx_pipe(osb, gsb, npart, tag):
                    pfw = pt(f"pfw{tag}", (npart, 392))
                    nc.vector.tensor_tensor(out=pfw[:], in0=osb[:], in1=gsb[:], op=AluOpType.add)
                    fiw = pt(f"fiw{tag}", (npart, 392), I16)
                    nc.vector.tensor_copy(out=fiw[:], in_=pfw[:])
                    frw = pt(f"frw{tag}", (npart, 392))
                    nc.vector.tensor_copy(out=frw[:], in_=fiw[:])
                    ddw = pt(f"ddw{tag}", (npart, 392))
                    nc.vector.tensor_tensor(out=ddw[:], in0=pfw[:], in1=frw[:], op=AluOpType.subtract)
                    ngw = pt(f"ngw{tag}", (npart, 392))
                    nc.vector.tensor_scalar(out=ngw[:], in0=ddw[:], scalar1=0.0, scalar2=None, op0=AluOpType.is_lt)
                    fw = pt(f"fw{tag}", (npart, 392))
                    nc.vector.tensor_tensor(out=fw[:], in0=frw[:], in1=ngw[:], op=AluOpType.subtract)
                    cw = pt(f"cw{tag}", (npart, 392))
                    nc.vector.tensor_scalar(out=cw[:], in0=fw[:], scalar1=7.0, scalar2=64.0,
                                            op0=AluOpType.max, op1=AluOpType.min)
                    jf = pt(f"jf{tag}", (npart, 196))
                    nc.vector.scalar_tensor_tensor(out=jf[:], in0=cw[:, 0:196], scalar=58.0,
                                                   in1=cw[:, 196:392], op0=AluOpType.mult, op1=AluOpType.add)
                    jf2 = pt(f"jf2{tag}", (npart, 196))
                    nc.vector.tensor_scalar(out=jf2[:], in0=jf[:], scalar1=-413.0, scalar2=None, op0=AluOpType.add)
                    ji = pt(f"ji{tag}", (npart, 196), I16)
                    nc.vector.tensor_copy(out=ji[:], in_=jf2[:])
                    return ji

                jiA = idx_pipe(owa, gwa, 128, "A")
                jiB = idx_pipe(owb, gwb, 16, "B")

                # assemble idx_all [128, KK, NCOL] i16 via 72 replication DMAs (HWDGE)
                idx_all = ctile([128, KK, NCOL], I16, "idx_all")
                nc.vector.memset(idx_all[:, :, NCOL_REAL:NCOL], 0)
                for k in range(KK):
                    src = jiA[16 * k:16 * k + 16, :] if k < 8 else jiB[:, :]
                    for g in range(8):
                        nc.sync.dma_start(out=idx_all[16 * g:16 * g + 16, k, 0:NCOL_REAL], in_=src)

                # ---- natural-layout weight pipeline ----
                pf = pt("pf")
                nc.vector.tensor_tensor(out=pf[:], in0=onat[:], in1=gnat[:], op=AluOpType.add)
                fi = pt("fi", (128, 448), I16)
                nc.vector.tensor_copy(out=fi[:], in_=pf[:])
                fr = pt("fr")
                nc.vector.tensor_copy(out=fr[:], in_=fi[:])
                dd = pt("dd")
                nc.vector.tensor_tensor(out=dd[:], in0=pf[:], in1=fr[:], op=AluOpType.subtract)
                ng = pt("ng")
                nc.vector.tensor_scalar(out=ng[:], in0=dd[:], scalar1=0.0, scalar2=None, op0=AluOpType.is_lt)
                ff = pt("ff")
                nc.vector.tensor_tensor(out=ff[:], in0=fr[:], in1=ng[:], op=AluOpType.subtract)
                tt = pt("tt")
                nc.vector.tensor_tensor(out=tt[:], in0=pf[:], in1=ff[:], op=AluOpType.subtract)
                a0 = pt("a0")
                nc.vector.tensor_scalar(out=a0[:], in0=ff[:], scalar1=8.0, scalar2=None, op0=AluOpType.is_ge)
                b0 = pt("b0")
                nc.vector.tensor_scalar(out=b0[:], in0=ff[:], scalar1=63.0, scalar2=None, op0=AluOpType.is_le)
                m0 = pt("m0")
                nc.vector.tensor_tensor(out=m0[:], in0=a0[:], in1=b0[:], op=AluOpType.mult)
                a1 = pt("a1")
                nc.vector.tensor_scalar(out=a1[:], in0=ff[:], scalar1=7.0, scalar2=None, op0=AluOpType.is_ge)
                b1 = pt("b1")
                nc.vector.tensor_scalar(out=b1[:], in0=ff[:], scalar1=62.0, scalar2=None, op0=AluOpType.is_le)
                m1 = pt("m1")
                nc.vector.tensor_tensor(out=m1[:], in0=a1[:], in1=b1[:], op=AluOpType.mult)
                onemt = pt("onemt")
                nc.vector.tensor_scalar(out=onemt[:], in0=tt[:], scalar1=1.0, scalar2=-1.0,
                                        op0=AluOpType.subtract, op1=AluOpType.mult)
                w0 = pt("w0")
                nc.vector.tensor_tensor(out=w0[:], in0=onemt[:], in1=m0[:], op=AluOpType.mult)
                w1 = pt("w1")
                nc.vector.tensor_tensor(out=w1[:], in0=tt[:], in1=m1[:], op=AluOpType.mult)

                # bring x-half weight planes to partition base 0 (walrus: TT needs equal bases)
                w0x = pt("w0x", (63, 448))
                nc.vector.tensor_copy(out=w0x[:], in_=w0[64:127, :])
                w1x = pt("w1x", (63, 448))
                nc.vector.tensor_copy(out=w1x[:], in_=w1[64:127, :])

                # products [63,448] f32 -> cast f16, order (w00, w10), (w01, w11)
                wprod = []
                for nm, (wy, wx) in (("w00", (w0, w0x)), ("w10", (w1, w0x)),
                                     ("w01", (w0, w1x)), ("w11", (w1, w1x))):
                    t = pt(nm, (63, 448))
                    nc.vector.tensor_tensor(out=t[:], in0=wy[0:63, :], in1=wx[:], op=AluOpType.mult)
                    th = pt(nm + "h", (63, 448), F16)
                    nc.vector.tensor_copy(out=th[:], in_=t[:])
                    wprod.append(th)

                # ---- per-quarter: flatten weights, gather, compute ----
                t2full = t2.ap()
                t2view = bass.AP(tensor=t2full.tensor, offset=t2full.offset,
                                 ap=[[128, TVIEW], [1, 256]])

                QUARTERS = [(0, 896), (896, 896), (1792, 896), (2688, 512)]
                for (qb, qn) in QUARTERS:
                    gtiles = {}
                    for k in range(KK):
                        g = gpool.tile([128, 2, qn], F16, tag="g", name="g")
                        nc.gpsimd.dma_gather(
                            g[:], t2view,
                            idx_all[:, k, qb // 16:qb // 16 + qn // 16],
                            qn, qn, 256, elem_step=128, transpose=True)
                        gtiles[k] = g

                    r0 = qb // 448   # first 448-row of the quarter
                    nrow = qn // 448 if qn % 448 == 0 else qn // 448 + 1  # 2 or 2(512->1+pad)
                    rep0 = const.tile([2, KK, qn], F16, tag="rep0", name="rep0", bufs=1)
                    rep1 = const.tile([2, KK, qn], F16, tag="rep1", name="rep1", bufs=1)
                    reps = (rep0, rep1)
                    nflat = min(448 * (7 - r0), qn)   # real (non-pad) cols
                    if nflat < qn:
                        nc.vector.memset(rep0[:, :, nflat:qn], 0.0)
                        nc.vector.memset(rep1[:, :, nflat:qn], 0.0)
                    for i, t in enumerate(wprod):
                        dst = reps[i // 2]
                        part = i % 2
                        for k in range(KK):
                            nc.sync.dma_start(out=dst[part:part + 1, k, 0:nflat],
                                              in_=t[7 * k + r0:7 * k + r0 + nflat // 448, :])

                    out_ps = pso.tile([OC, qn], F32, tag="out_ps", name="out_ps")
                    chunks = [(0, 512), (512, 384)] if qn == 896 else [(0, 512)]
                    for (c0, cn) in chunks:
                        for k in range(KK):
                            g = gtiles[k]
                            wt_ps = psw.tile([128, 2, cn], F32, tag="wtps", name="wtps", padded_shape=[128, 2, 512])
                            for ss in range(2):
                                nc.tensor.matmul(wt_ps[:, ss, :], rlhs_sb[:],
                                                 reps[ss][:, k, c0:c0 + cn],
                                                 start=True, stop=True)
                            wt_sb = wtpool.tile([128, 2, cn], F16, tag="wtsb", name="wtsb")
                            nc.scalar.copy(out=wt_sb[:], in_=wt_ps[:])
                            u = upool.tile([128, 2, cn], F16, tag="u", name="u")
                            nc.vector.tensor_tensor(out=u[:], in0=g[:, :, c0:c0 + cn],
                                                    in1=wt_sb[:], op=AluOpType.mult)
                            for ss in range(2):
                                nc.tensor.matmul(out_ps[:, c0:c0 + cn], wlhs_sb[:, k, :],
                                                 u[:, ss, :],
                                                 start=(k == 0 and ss == 0), stop=(k == KK - 1 and ss == 1))
                    nreal = min(qn, P - qb)
                    osb = opool.tile([OC, qn], F32, tag="osb", name="osb")
                    nc.vector.tensor_copy(out=osb[:], in_=out_ps[:])
                    nc.sync.dma_start(out=out.ap()[:, qb:qb + nreal], in_=osb[:, 0:nreal])


    nc.compile()
    return nc


# ---------------- runners ----------------

def np_reference(x, off, wt):
    """numpy replica of reference._deform_conv2d for one image."""
    Cc, Hh, Ww = x.shape
    off = off.reshape(KK, 2, Hh, Ww)
    gy, gx = _grids_f32()
    out = np.zeros((KK, Cc, Hh * Ww), dtype=np.float64)
    xf = x.reshape(Cc, -1)
    for k in range(KK):
        py = off[k, 0].reshape(-1) + gy[k] - 8.0
        px = off[k, 1].reshape(-1) + gx[k] - 8.0
        y0 = np.floor(py).astype(np.int64)
        x0 = np.floor(px).astype(np.int64)
        ty = py - y0
        tx = px - x0
        acc = np.zeros((Cc, Hh * Ww))
        for (dy, dx, wgt) in ((0, 0, (1 - ty) * (1 - tx)), (0, 1, (1 - ty) * tx),
                              (1, 0, ty * (1 - tx)), (1, 1, ty * tx)):
            cy, cx = y0 + dy, x0 + dx
            valid = (cy >= 0) & (cy < Hh) & (cx >= 0) & (cx < Ww)
            idx = np.clip(cy, 0, Hh - 1) * Ww + np.clip(cx, 0, Ww - 1)
            v = xf[:, idx] * valid[None]
            acc += v * wgt[None]
        out[k] = acc
    w9 = wt.reshape(OC, Cc, KK)
    return np.einsum('ock,kcp->op', w9, out).astype(np.float32)


def run(inputs: dict, trace=False):
    x = np.asarray(inputs["input"])
    off = np.asarray(inputs["offset"])
    wt = np.asarray(inputs["weight"])
    B = x.shape[0]
    nc = gen_kernel(B)
    in_maps = [host_inputs(x[b], off[b], wt) for b in range(B)]
    res = run_bass_kernel_spmd(nc, in_maps, core_ids=list(range(B)), trace=trace)
    outs = np.stack([np.asarray(r["out"]).reshape(OC, H, W) for r in res.results])
    return outs.astype(np.float32), res


# ---------------- graded entry point ----------------

LAST_EXEC_NS = None

def kernel(input, offset, weight):
    """Full-batch DeformConv2d on 8 NeuronCores (batch-parallel)."""
    x = np.asarray(input, dtype=np.float32)
    off = np.asarray(offset, dtype=np.float32)
    wt = np.asarray(weight, dtype=np.float32)
    B = x.shape[0]
    nc = gen_kernel(B)
    in_maps = [host_inputs(x[b], off[b], wt) for b in range(B)]
    res = run_bass_kernel_spmd(nc, in_maps, core_ids=list(range(B)))
    global LAST_EXEC_NS
    LAST_EXEC_NS = res.exec_time_ns
    out = np.stack([np.asarray(r["out"]).reshape(OC, H, W) for r in res.results])
    return out.astype(np.float32)

